# revision 2
# baseline (speedup 1.0000x reference)
"""Trainium2 Bass kernel for nn_DistortionLossDisparity (8-core SPMD).

Math: the reference's column gather `m` is a row-wise permutation of
T = t@t.T, and log-softmax's LSE is permutation-invariant, so

    loss = mean_i [ logsumexp_k(10*|t_i.t_k - s_i|) - 10*|s_i - t_i.t_c(i)| ]

with s_i = q_i . q_{j_i} and c(i) = m[i, label_i] (a single index per row,
computed on the host from j_idx/labels). Each of the 8 cores handles 1024
rows: PE computes T row-blocks (f32r matmuls) into PSUM, one fused custom
DVE op produces |T - s| in SBUF plus the running row max, one ACT pass does
exp(10x - 10M) with fused row-sum accumulation. Host sums 8x[128] partials.
"""
import os
import sys

for _p in ("/opt/trn_rl_repo", os.path.expanduser("~/.axon_site/_ro/trn_rl_repo")):
    if os.path.isdir(_p) and _p not in sys.path:
        sys.path.insert(0, _p)

import numpy as np

N, D = 8192, 128
P = 128
N_CORES = 8
ROWS_PER_CORE = N // N_CORES          # 1024
BLOCKS = ROWS_PER_CORE // P           # 8
CHUNK = 2048                          # PSUM chunk: 4 banks of 512 fp32
CHUNKS = N // CHUNK                   # 4 chunks per row-block
INV_TEMP = 10.0                       # 1 / 0.1


# --------------------------------------------------------------------------
# custom DVE op:  out = |in0 - s0|,  accum_out = max_k out   (one pass)
# --------------------------------------------------------------------------
def _register_abs_sub_max():
    import concourse.dve_ops as dve_ops
    from concourse.dve_ops import DveOp
    from concourse.dve_spec import Spec, Src0, C0, maxx, AluOp, lower, Zero, _has_src1
    from concourse.dve_uop import DveOpSpec

    name = "ABS_SUB_MAX_ANT"
    for op in dve_ops.OPS:
        if op.name == name:
            return op

    def _ref(in0, in1, s0, s1, imm2):
        out = np.abs(in0.astype(np.float32) - s0)
        return out, out.reshape(out.shape[0], -1).max(axis=-1, keepdims=True)

    d = Src0 - C0
    spec = Spec(body=maxx(d, -d), accum=AluOp.MAX, accum_init=Zero, reference=_ref)

    opcode = dve_ops._CUSTOM_DVE_ROW_BASE + len(dve_ops.OPS)
    assert opcode < 0x20
    shas = {}
    for ver in ("v3", "v4"):
        s = DveOpSpec(name=name, opcode=opcode, uops=lower(spec, ver=ver),
                      rd1_en=_has_src1(spec))
        shas[ver] = s.sha(ver)

    op = DveOp(name, spec, subdim=False, uops_sha=shas)
    dve_ops.OPS.append(op)
    dve_ops._SUB_OPCODE_FOR_NAME[name] = opcode
    dve_ops.CUSTOM_DVE_SPECS[name] = spec
    return op


# --------------------------------------------------------------------------
# device program (built once, cached)
# --------------------------------------------------------------------------
_CACHED_NC = None


def _build_nc():
    global _CACHED_NC
    if _CACHED_NC is not None:
        return _CACHED_NC

    from contextlib import ExitStack
    from concourse import bacc, tile, mybir

    abs_sub_max = _register_abs_sub_max()
    import concourse.dve_ops as dve_ops

    f32 = mybir.dt.float32
    f32r = mybir.dt.float32r
    bf16 = mybir.dt.bfloat16

    nc = bacc.Bacc("TRN2", target_bir_lowering=False, debug=False,
                   enable_asserts=True, num_devices=N_CORES)

    tT_d = nc.dram_tensor("tT", [P, N], f32, kind="ExternalInput").ap()
    tTblk_d = nc.dram_tensor("tTblk", [P, ROWS_PER_CORE], f32, kind="ExternalInput").ap()
    q_d = nc.dram_tensor("q_sh", [P, ROWS_PER_CORE], f32, kind="ExternalInput").ap()
    qj_d = nc.dram_tensor("qj_sh", [P, ROWS_PER_CORE], f32, kind="ExternalInput").ap()
    t_d = nc.dram_tensor("t_sh", [P, ROWS_PER_CORE], f32, kind="ExternalInput").ap()
    tc_d = nc.dram_tensor("tc_sh", [P, ROWS_PER_CORE], f32, kind="ExternalInput").ap()
    out_d = nc.dram_tensor("partials", [P, 1], f32, kind="ExternalOutput").ap()

    with tile.TileContext(nc, trace_sim=False) as tc, ExitStack() as ctx:
        const = ctx.enter_context(tc.tile_pool(name="const", bufs=1))
        work = ctx.enter_context(tc.tile_pool(name="work", bufs=2))
        ps = ctx.enter_context(tc.tile_pool(name="ps", bufs=2, space="PSUM"))

        tT_s = const.tile([P, N], f32r)
        tTblk_s = const.tile([P, ROWS_PER_CORE], f32r)
        q_s = const.tile([P, ROWS_PER_CORE], f32)
        qj_s = const.tile([P, ROWS_PER_CORE], f32)
        t_s = const.tile([P, ROWS_PER_CORE], f32)
        tc_s = const.tile([P, ROWS_PER_CORE], f32)
        nc.sync.dma_start(out=tT_s[:], in_=tT_d[:].bitcast(f32r))
        nc.sync.dma_start(out=tTblk_s[:], in_=tTblk_d[:].bitcast(f32r))
        nc.sync.dma_start(out=q_s[:], in_=q_d[:])
        nc.sync.dma_start(out=qj_s[:], in_=qj_d[:])
        nc.sync.dma_start(out=t_s[:], in_=t_d[:])
        nc.sync.dma_start(out=tc_s[:], in_=tc_d[:])

        s_sh = const.tile([P, BLOCKS], f32)     # s_i per (partition, block)
        d_sh = const.tile([P, BLOCKS], f32)     # t_i . t_c(i)
        Mall = const.tile([P, BLOCKS], f32)     # row maxes of |T - s|
        Sall = const.tile([P, BLOCKS], f32)     # row sums of exp
        dummy = const.tile([P, P], f32)         # discarded custom-op body out
        dummy1 = const.tile([P, 1], f32)

        # rowwise dots: s = sum(q*qj), d = sum(t*tc) per block
        for b in range(BLOCKS):
            cs = slice(P * b, P * (b + 1))
            nc.vector._custom_dve(
                dve_ops.TENSOR_TENSOR_REDUCE,
                out=dummy[:], in0=q_s[:, cs], in1=qj_s[:, cs],
                s0=0.0, s1=1.0, accum_out=s_sh[:, b:b + 1])
            nc.vector._custom_dve(
                dve_ops.TENSOR_TENSOR_REDUCE,
                out=dummy[:], in0=t_s[:, cs], in1=tc_s[:, cs],
                s0=0.0, s1=1.0, accum_out=d_sh[:, b:b + 1])

        for b in range(BLOCKS):
            ablock = work.tile([P, N], f32, tag="ablock")
            mparts = work.tile([P, CHUNKS], f32, tag="mparts")
            lhsT = tTblk_s[:, P * b:P * (b + 1)]
            for c in range(CHUNKS):
                psum = ps.tile([P, CHUNK], f32, tag="psum")
                for k in range(CHUNK // 512):
                    col = CHUNK * c + 512 * k
                    nc.tensor.matmul(
                        out=psum[:, 512 * k:512 * (k + 1)],
                        lhsT=lhsT, rhs=tT_s[:, col:col + 512],
                        start=True, stop=True)
                nc.vector._custom_dve(
                    abs_sub_max,
                    out=ablock[:, CHUNK * c:CHUNK * (c + 1)],
                    in0=psum[:], s0=s_sh[:, b:b + 1],
                    accum_out=mparts[:, c:c + 1])

            # M_b = max over chunk partials; bias = -10*M_b
            nc.vector.tensor_reduce(out=Mall[:, b:b + 1], in_=mparts[:],
                                    axis=mybir.AxisListType.X,
                                    op=mybir.AluOpType.max)
            bias_b = work.tile([P, 1], f32, tag="bias")
            nc.vector.tensor_scalar(out=bias_b[:], in0=Mall[:, b:b + 1],
                                    scalar1=-INV_TEMP, scalar2=None,
                                    op0=mybir.AluOpType.mult)
            scratch = work.tile([P, N], bf16, tag="scratch")
            nc.scalar.activation(out=scratch[:], in_=ablock[:],
                                 func=mybir.ActivationFunctionType.Exp,
                                 bias=bias_b[:], scale=INV_TEMP,
                                 accum_out=Sall[:, b:b + 1])

        # tail: loss_rows = 10*M + log(S) - 10*|s - d|  summed over blocks
        logS = const.tile([P, BLOCKS], f32)
        nc.scalar.activation(out=logS[:], in_=Sall[:],
                             func=mybir.ActivationFunctionType.Ln)
        dterm = const.tile([P, BLOCKS], f32)
        for b in range(BLOCKS):
            nc.vector._custom_dve(
                abs_sub_max,
                out=dterm[:, b:b + 1], in0=d_sh[:, b:b + 1],
                s0=s_sh[:, b:b + 1], accum_out=dummy1[:])
        m10 = const.tile([P, BLOCKS], f32)
        nc.vector.tensor_scalar(out=m10[:], in0=Mall[:], scalar1=INV_TEMP,
                                scalar2=None, op0=mybir.AluOpType.mult)
        dt10 = const.tile([P, BLOCKS], f32)
        nc.vector.tensor_scalar(out=dt10[:], in0=dterm[:], scalar1=-INV_TEMP,
                                scalar2=None, op0=mybir.AluOpType.mult)
        lrows = const.tile([P, BLOCKS], f32)
        nc.vector.tensor_add(lrows[:], m10[:], logS[:])
        nc.vector.tensor_add(lrows[:], lrows[:], dt10[:])
        partial = const.tile([P, 1], f32)
        nc.vector.tensor_reduce(out=partial[:], in_=lrows[:],
                                axis=mybir.AxisListType.X,
                                op=mybir.AluOpType.add)
        nc.sync.dma_start(out=out_d[:], in_=partial[:])

    nc.compile()
    _CACHED_NC = nc
    return nc


def _layout(x):
    """[1024, 128] row-shard -> [128 partitions, 1024] block-major layout."""
    return np.ascontiguousarray(
        x.reshape(BLOCKS, P, D).transpose(1, 0, 2).reshape(P, ROWS_PER_CORE))


def _make_in_maps(q, t, labels, j_idx):
    i = np.arange(N, dtype=np.int64)
    j = j_idx.astype(np.int64)
    l = labels.astype(np.int64)
    # column index c(i) = m[i, labels[i]] per the reference's neg_ts mapping
    col = np.where(
        l == i, j,
        np.where(j > i,
                 np.where((l > i) & (l <= j), l - 1, l),
                 np.where((l >= j) & (l < i), l + 1, l)))

    tT = np.ascontiguousarray(t.T)  # [128, 8192]
    qj = q[j]
    tcol = t[col]

    in_maps = []
    for c in range(N_CORES):
        rs = slice(ROWS_PER_CORE * c, ROWS_PER_CORE * (c + 1))
        in_maps.append({
            "tT": tT,
            "tTblk": np.ascontiguousarray(tT[:, rs]),
            "q_sh": _layout(q[rs]),
            "qj_sh": _layout(qj[rs]),
            "t_sh": _layout(t[rs]),
            "tc_sh": _layout(tcol[rs]),
        })
    return in_maps


def _run(inputs, trace=False):
    from concourse.bass_utils import run_bass_kernel_spmd

    q = np.asarray(inputs["q_seed_features_sampled"], dtype=np.float32)
    t = np.asarray(inputs["t_seed_features_sampled"], dtype=np.float32)
    labels = np.asarray(inputs["cl_loss_label"])
    j_idx = np.asarray(inputs["j_idx"])
    assert q.shape == (N, D) and t.shape == (N, D)

    nc = _build_nc()
    in_maps = _make_in_maps(q, t, labels, j_idx)
    res = run_bass_kernel_spmd(nc, in_maps, list(range(N_CORES)), trace=trace)
    total = np.float64(0.0)
    for r in res.results:
        total += r["partials"].astype(np.float64).sum()
    loss = np.array(total / N, dtype=np.float32)
    return loss, res


def kernel(**inputs) -> np.ndarray:
    loss, _ = _run(inputs, trace=False)
    return loss


# revision 11
# speedup vs baseline: 8837.7730x; 8837.7730x over previous
"""Trainium2 Bass kernel for nn_DistortionLossDisparity (8-core SPMD).

Math: the reference's column gather `m` is a row-wise permutation of
T = t@t.T, and log-softmax's LSE is permutation-invariant, so

    loss = mean_i [ logsumexp_k(10*|t_i.t_k - s_i|) - 10*|s_i - t_i.t_c(i)| ]

with s_i = q_i . q_{j_i} and c(i) = m[i, label_i] (a single index per row,
computed on the host from j_idx/labels). Each of the 8 cores handles 1024
rows: PE computes T row-blocks (f32r matmuls) into PSUM, one fused custom
DVE op produces |T - s| in SBUF plus the running row max, one ACT pass does
exp(10x - 10M) with fused row-sum accumulation. Host sums 8x[128] partials.
"""
import os
import sys

for _p in ("/opt/trn_rl_repo", os.path.expanduser("~/.axon_site/_ro/trn_rl_repo")):
    if os.path.isdir(_p) and _p not in sys.path:
        sys.path.insert(0, _p)

import numpy as np

N, D = 8192, 128
P = 128
N_CORES = 8
ROWS_PER_CORE = N // N_CORES          # 1024
BLOCKS = ROWS_PER_CORE // P           # 8
CHUNK = 2048                          # PSUM chunk: 4 banks of 512 fp32
CHUNKS = N // CHUNK                   # 4 chunks per row-block
INV_TEMP = 10.0                       # 1 / 0.1


# --------------------------------------------------------------------------
# custom DVE op:  out = |in0 - s0|,  accum_out = max_k out   (one pass)
# --------------------------------------------------------------------------
def _register_abs_sub_max():
    import concourse.dve_ops as dve_ops
    from concourse.dve_ops import DveOp
    from concourse.dve_spec import Spec, Src0, C0, maxx, AluOp, lower, Zero, _has_src1
    from concourse.dve_uop import DveOpSpec

    name = "ABS_SUB_MAX_ANT"
    for op in dve_ops.OPS:
        if op.name == name:
            return op

    def _ref(in0, in1, s0, s1, imm2):
        out = np.abs(in0.astype(np.float32) - s0)
        return out, out.reshape(out.shape[0], -1).max(axis=-1, keepdims=True)

    d = Src0 - C0
    spec = Spec(body=maxx(d, -d), accum=AluOp.MAX, accum_init=Zero, reference=_ref)

    opcode = dve_ops._CUSTOM_DVE_ROW_BASE + len(dve_ops.OPS)
    assert opcode < 0x20
    shas = {}
    for ver in ("v3", "v4"):
        s = DveOpSpec(name=name, opcode=opcode, uops=lower(spec, ver=ver),
                      rd1_en=_has_src1(spec))
        shas[ver] = s.sha(ver)

    op = DveOp(name, spec, subdim=False, uops_sha=shas)
    dve_ops.OPS.append(op)
    dve_ops._SUB_OPCODE_FOR_NAME[name] = opcode
    dve_ops.CUSTOM_DVE_SPECS[name] = spec
    return op


def _register_neg10_abs_sub_min():
    """out = -10*|in0 - s0|, accum_out = min_k out = -10*max|in0 - s0|.
    The accum is directly usable as the ACT exp bias: exp(-x + bias)."""
    import concourse.dve_ops as dve_ops
    from concourse.dve_ops import DveOp
    from concourse.dve_spec import Spec, Src0, C0, C2, minn, AluOp, lower, Zero, _has_src1
    from concourse.dve_uop import DveOpSpec

    name = "NEG10_ABS_SUB_MIN_ANT"
    for op in dve_ops.OPS:
        if op.name == name:
            return op

    def _ref(in0, in1, s0, s1, imm2):
        out = imm2 * np.abs(in0.astype(np.float32) - s0) * -1.0
        return out, out.reshape(out.shape[0], -1).min(axis=-1, keepdims=True)

    e = (Src0 - C0) * C2
    spec = Spec(body=minn(e, -e), accum=AluOp.MIN, accum_init=Zero, reference=_ref)

    opcode = dve_ops._CUSTOM_DVE_ROW_BASE + len(dve_ops.OPS)
    assert opcode < 0x20
    shas = {}
    for ver in ("v3", "v4"):
        s = DveOpSpec(name=name, opcode=opcode, uops=lower(spec, ver=ver),
                      rd1_en=_has_src1(spec))
        shas[ver] = s.sha(ver)

    op = DveOp(name, spec, subdim=False, uops_sha=shas)
    dve_ops.OPS.append(op)
    dve_ops._SUB_OPCODE_FOR_NAME[name] = opcode
    dve_ops.CUSTOM_DVE_SPECS[name] = spec
    return op


# --------------------------------------------------------------------------
# device program
# --------------------------------------------------------------------------
def build_nc(reps: int = 1, ablock_bufs: int = 3, dma_split: int = 8, variant: str = 'full'):
    """Build + bacc-compile the SPMD program. reps>1 wraps the compute body
    in a For_i loop (benchmarking only)."""
    from contextlib import ExitStack
    from concourse import bacc, tile, mybir

    abs_sub_max = _register_abs_sub_max()
    neg10_op = _register_neg10_abs_sub_min()
    import concourse.dve_ops as dve_ops

    f32 = mybir.dt.float32
    f32r = mybir.dt.float32r
    bf16 = mybir.dt.bfloat16

    nc = bacc.Bacc("TRN2", target_bir_lowering=False, debug=False,
                   enable_asserts=True, num_devices=N_CORES)

    tT_d = nc.dram_tensor("tT", [P, N], f32, kind="ExternalInput").ap()
    tTblk_d = nc.dram_tensor("tTblk", [P, ROWS_PER_CORE], f32, kind="ExternalInput").ap()
    q_d = nc.dram_tensor("q_sh", [P, ROWS_PER_CORE], f32, kind="ExternalInput").ap()
    qj_d = nc.dram_tensor("qj_sh", [P, ROWS_PER_CORE], f32, kind="ExternalInput").ap()
    t_d = nc.dram_tensor("t_sh", [P, ROWS_PER_CORE], f32, kind="ExternalInput").ap()
    tc_d = nc.dram_tensor("tc_sh", [P, ROWS_PER_CORE], f32, kind="ExternalInput").ap()
    out_d = nc.dram_tensor("partials", [P, 1], f32, kind="ExternalOutput").ap()

    with tile.TileContext(nc, trace_sim=False) as tc, ExitStack() as ctx:
        const = ctx.enter_context(tc.tile_pool(name="const", bufs=1))
        work = ctx.enter_context(tc.tile_pool(name="work", bufs=2))
        apool = ctx.enter_context(tc.tile_pool(name="apool", bufs=ablock_bufs))
        ps = ctx.enter_context(tc.tile_pool(name="ps", bufs=2, space="PSUM"))

        tT_s = const.tile([P, N], f32r)
        tTblk_s = const.tile([P, ROWS_PER_CORE], f32r)
        q_s = const.tile([P, ROWS_PER_CORE], f32)
        qj_s = const.tile([P, ROWS_PER_CORE], f32)
        t_s = const.tile([P, ROWS_PER_CORE], f32)
        tc_s = const.tile([P, ROWS_PER_CORE], f32)
        step = N // dma_split
        for i in range(dma_split):
            cs = slice(step * i, step * (i + 1))
            nc.sync.dma_start(out=tT_s[:, cs], in_=tT_d[:, cs].bitcast(f32r))
        nc.sync.dma_start(out=tTblk_s[:], in_=tTblk_d[:].bitcast(f32r))
        nc.sync.dma_start(out=q_s[:], in_=q_d[:])
        nc.sync.dma_start(out=qj_s[:], in_=qj_d[:])
        nc.sync.dma_start(out=t_s[:], in_=t_d[:])
        nc.sync.dma_start(out=tc_s[:], in_=tc_d[:])

        s_sh = const.tile([P, BLOCKS], f32)     # s_i per (partition, block)
        d_sh = const.tile([P, BLOCKS], f32)     # t_i . t_c(i)
        Mall = const.tile([P, BLOCKS], f32)     # row maxes of |T - s|
        Sall = const.tile([P, BLOCKS], f32)     # row sums of exp
        dummy = const.tile([P, P], f32)         # discarded custom-op body out
        dummy1 = const.tile([P, 1], f32)

        def body(_i=None):
            # rowwise dots: s = sum(q*qj) per block (d deferred to the tail)
            for b in range(BLOCKS):
                cs = slice(P * b, P * (b + 1))
                nc.vector._custom_dve(
                    dve_ops.TENSOR_TENSOR_REDUCE,
                    out=dummy[:], in0=q_s[:, cs], in1=qj_s[:, cs],
                    s0=0.0, s1=1.0, accum_out=s_sh[:, b:b + 1])

            for b in range(BLOCKS):
                ablock = apool.tile([P, N], f32, tag="ablock")
                mparts = work.tile([P, CHUNKS], f32, tag="mparts")
                if variant == "full2":
                    Scs = work.tile([P, CHUNKS], f32, tag="Scs")
                    scratch2 = work.tile([P, N], bf16, tag="scratch")
                lhsT = tTblk_s[:, P * b:P * (b + 1)]
                for c in range(CHUNKS):
                    psum = ps.tile([P, CHUNK], f32, tag="psum")
                    for k in range(CHUNK // 512):
                        col = CHUNK * c + 512 * k
                        nc.tensor.matmul(
                            out=psum[:, 512 * k:512 * (k + 1)],
                            lhsT=lhsT, rhs=tT_s[:, col:col + 512],
                            start=True, stop=True)
                    if variant == "full2":
                        # out = -10|T - s|, accum = -10 * chunk max
                        nc.vector._custom_dve(
                            neg10_op,
                            out=ablock[:, CHUNK * c:CHUNK * (c + 1)],
                            in0=psum[:], s0=s_sh[:, b:b + 1], imm2=INV_TEMP,
                            accum_out=mparts[:, c:c + 1])
                        # chunk-local exp: exp(10|a| - 10 m_c)
                        nc.scalar.activation(
                            out=scratch2[:, CHUNK * c:CHUNK * (c + 1)],
                            in_=ablock[:, CHUNK * c:CHUNK * (c + 1)],
                            func=mybir.ActivationFunctionType.Exp,
                            bias=mparts[:, c:c + 1], scale=-1.0,
                            accum_out=Scs[:, c:c + 1])
                    elif variant != "pe":
                        nc.vector._custom_dve(
                            abs_sub_max,
                            out=ablock[:, CHUNK * c:CHUNK * (c + 1)],
                            in0=psum[:], s0=s_sh[:, b:b + 1],
                            accum_out=mparts[:, c:c + 1])

                if variant == "pe":
                    nc.vector.tensor_scalar(out=Mall[:, b:b + 1], in0=dummy1[:],
                                            scalar1=1.0, scalar2=None,
                                            op0=mybir.AluOpType.mult)
                elif variant == "full2":
                    # Mall holds amin_b = -10*M_b
                    nc.vector.tensor_reduce(out=Mall[:, b:b + 1], in_=mparts[:],
                                            axis=mybir.AxisListType.X,
                                            op=mybir.AluOpType.min)
                    # w_c = exp(10 m_c - 10 M_b) = exp(-mparts_c + amin_b)
                    w4 = work.tile([P, CHUNKS], f32, tag="w4")
                    nc.scalar.activation(out=w4[:], in_=mparts[:],
                                         func=mybir.ActivationFunctionType.Exp,
                                         bias=Mall[:, b:b + 1], scale=-1.0)
                    # S_b = sum_c Sc * w_c
                    nc.vector._custom_dve(
                        dve_ops.TENSOR_TENSOR_REDUCE,
                        out=dummy[:, 0:CHUNKS], in0=Scs[:], in1=w4[:],
                        s0=0.0, s1=1.0, accum_out=Sall[:, b:b + 1])
                else:
                    nc.vector.tensor_reduce(out=Mall[:, b:b + 1], in_=mparts[:],
                                            axis=mybir.AxisListType.X,
                                            op=mybir.AluOpType.max)
                if variant == "full":
                    bias_b = work.tile([P, 1], f32, tag="bias")
                    nc.vector.tensor_scalar(out=bias_b[:], in0=Mall[:, b:b + 1],
                                            scalar1=-INV_TEMP, scalar2=None,
                                            op0=mybir.AluOpType.mult)
                    scratch = work.tile([P, N], bf16, tag="scratch")
                    nc.scalar.activation(out=scratch[:], in_=ablock[:],
                                         func=mybir.ActivationFunctionType.Exp,
                                         bias=bias_b[:], scale=INV_TEMP,
                                         accum_out=Sall[:, b:b + 1])
                else:
                    nc.vector.tensor_scalar(out=Sall[:, b:b + 1], in0=dummy1[:],
                                            scalar1=1.0, scalar2=None,
                                            op0=mybir.AluOpType.mult)

        if reps > 1:
            with tc.For_i(0, reps, 1) as i:
                body(i)
        else:
            body()

        # tail: loss_rows = 10*M + log(S) - 10*|s - d| summed over blocks
        for b in range(BLOCKS):
            cs = slice(P * b, P * (b + 1))
            nc.vector._custom_dve(
                dve_ops.TENSOR_TENSOR_REDUCE,
                out=dummy[:], in0=t_s[:, cs], in1=tc_s[:, cs],
                s0=0.0, s1=1.0, accum_out=d_sh[:, b:b + 1])
        logS = const.tile([P, BLOCKS], f32)
        nc.scalar.activation(out=logS[:], in_=Sall[:],
                             func=mybir.ActivationFunctionType.Ln)
        dterm = const.tile([P, BLOCKS], f32)
        for b in range(BLOCKS):
            nc.vector._custom_dve(
                abs_sub_max,
                out=dterm[:, b:b + 1], in0=d_sh[:, b:b + 1],
                s0=s_sh[:, b:b + 1], accum_out=dummy1[:])
        m10 = const.tile([P, BLOCKS], f32)
        nc.vector.tensor_scalar(out=m10[:], in0=Mall[:],
                                scalar1=(-1.0 if variant == "full2" else INV_TEMP),
                                scalar2=None, op0=mybir.AluOpType.mult)
        dt10 = const.tile([P, BLOCKS], f32)
        nc.vector.tensor_scalar(out=dt10[:], in0=dterm[:], scalar1=-INV_TEMP,
                                scalar2=None, op0=mybir.AluOpType.mult)
        lrows = const.tile([P, BLOCKS], f32)
        nc.vector.tensor_add(lrows[:], m10[:], logS[:])
        nc.vector.tensor_add(lrows[:], lrows[:], dt10[:])
        partial = const.tile([P, 1], f32)
        nc.vector.tensor_reduce(out=partial[:], in_=lrows[:],
                                axis=mybir.AxisListType.X,
                                op=mybir.AluOpType.add)
        nc.sync.dma_start(out=out_d[:], in_=partial[:])

    nc.compile()
    return nc


_CACHED_NC = None


def _build_nc():
    global _CACHED_NC
    if _CACHED_NC is None:
        _CACHED_NC = build_nc()
    return _CACHED_NC


def _layout(x):
    """[1024, 128] row-shard -> [128 partitions, 1024] block-major layout."""
    return np.ascontiguousarray(
        x.reshape(BLOCKS, P, D).transpose(1, 0, 2).reshape(P, ROWS_PER_CORE))


def _make_in_maps(q, t, labels, j_idx):
    i = np.arange(N, dtype=np.int64)
    j = j_idx.astype(np.int64)
    l = labels.astype(np.int64)
    # column index c(i) = m[i, labels[i]] per the reference's neg_ts mapping
    col = np.where(
        l == i, j,
        np.where(j > i,
                 np.where((l > i) & (l <= j), l - 1, l),
                 np.where((l >= j) & (l < i), l + 1, l)))

    tT = np.ascontiguousarray(t.T)  # [128, 8192]
    qj = q[j]
    tcol = t[col]

    in_maps = []
    for c in range(N_CORES):
        rs = slice(ROWS_PER_CORE * c, ROWS_PER_CORE * (c + 1))
        in_maps.append({
            "tT": tT,
            "tTblk": np.ascontiguousarray(tT[:, rs]),
            "q_sh": _layout(q[rs]),
            "qj_sh": _layout(qj[rs]),
            "t_sh": _layout(t[rs]),
            "tc_sh": _layout(tcol[rs]),
        })
    return in_maps


def _run(inputs, trace=False):
    from concourse.bass_utils import run_bass_kernel_spmd

    q = np.asarray(inputs["q_seed_features_sampled"], dtype=np.float32)
    t = np.asarray(inputs["t_seed_features_sampled"], dtype=np.float32)
    labels = np.asarray(inputs["cl_loss_label"])
    j_idx = np.asarray(inputs["j_idx"])
    assert q.shape == (N, D) and t.shape == (N, D)

    nc = _build_nc()
    in_maps = _make_in_maps(q, t, labels, j_idx)
    res = run_bass_kernel_spmd(nc, in_maps, list(range(N_CORES)), trace=trace)
    total = np.float64(0.0)
    for r in res.results:
        total += r["partials"].astype(np.float64).sum()
    loss = np.array(total / N, dtype=np.float32)
    return loss, res


def kernel(**inputs) -> np.ndarray:
    loss, _ = _run(inputs, trace=False)
    return loss
